# revision 4
# baseline (speedup 1.0000x reference)
"""Trainium2 Bass kernel for nn_ARBlock (LN -> LSTM residual; LN -> MLP residual).

Strategy: pure data-parallel over batch (B=32 -> 4 examples/core on 8 cores,
no collectives).  Per core:
  Phase AB: LN1 + input-gate GEMM  xg.T = Wi'.T @ z.T   (bf16, written to DRAM)
  Phase C : sequential LSTM recurrence over S=2048 steps.  Per step the
            gates land transposed in PSUM: gates.T[m-tile] = sum_k Wh'[k,m].T @ h.T[k],
            with xg injected into PSUM first via an identity matmul so the
            Wh matmuls accumulate on top.  Gate math is sigmoid-only:
            the g-gate columns of Wi'/Wh' are pre-scaled by 2 on the host so
            tanh(g) = 2*sigmoid(2g) - 1, and the cell state is tracked as
            c' = 2c so tanh(c) = 2*sigmoid(c') - 1.
  Phase D : residual + LN2 + MLP (gelu-tanh), residual add, output.

Gate column order is permuted on the host to [i, g, o, f] so that i,g,o share
one PSUM bank (their sigmoid can start while the f-gate matmuls still run)
and f sits in a second bank.
"""

import numpy as np
import ml_dtypes

import concourse.bass as bass
import concourse.tile as tile
from concourse import bacc, mybir
from concourse.bass import ts, ds
from concourse.bass_utils import run_bass_kernel_spmd

AF = mybir.ActivationFunctionType
ALU = mybir.AluOpType
F32 = mybir.dt.float32
BF16 = mybir.dt.bfloat16

D = 512
F = 4 * D          # 2048 gate dim
KT = D // 128      # 4 k tiles
MT = F // 128      # 16 m tiles
B_LOC = 4          # batch per core
N_CORES = 8
EPS = 1e-6
UNROLL = 8         # recurrence steps per For_i body
TCH = 128          # recurrence steps per phase-AB/D chunk (512 tokens)


def _build(S):
    """Build the per-core Bass graph.  Returns compiled nc."""
    nc = bacc.Bacc(
        "TRN2",
        target_bir_lowering=False,
        debug=False,
        enable_asserts=False,
        num_devices=N_CORES,
    )
    TOK = B_LOC * S
    n_chunks = S // TCH

    xs = nc.dram_tensor("xs", [B_LOC, S, D], F32, kind="ExternalInput").ap()
    whp = nc.dram_tensor("whp", [128, KT, MT, 128], BF16, kind="ExternalInput").ap()
    wip = nc.dram_tensor("wip", [128, KT, MT, 128], BF16, kind="ExternalInput").ap()
    w1p = nc.dram_tensor("w1p", [128, KT, MT, 128], BF16, kind="ExternalInput").ap()
    w2p = nc.dram_tensor("w2p", [128, MT, KT, 128], BF16, kind="ExternalInput").ap()
    bi_d = nc.dram_tensor("bi", [128, MT], F32, kind="ExternalInput").ap()
    b1_d = nc.dram_tensor("b1", [128, MT], F32, kind="ExternalInput").ap()
    b2_d = nc.dram_tensor("b2", [128, KT], F32, kind="ExternalInput").ap()
    id_d = nc.dram_tensor("ident", [128, 128], BF16, kind="ExternalInput").ap()
    out = nc.dram_tensor("out", [B_LOC, S, D], F32, kind="ExternalOutput").ap()

    def x_tile_src(arr, c, q):
        # 128 tokens (32 t-steps x 4 batch), token index = 4*t + b
        t0 = c * TCH + q * 32
        return arr[:, t0:t0 + 32, :].transpose([1, 0, 2])

    with tile.TileContext(nc) as tc:
        with (
            tc.tile_pool(name="dram", bufs=1, space="DRAM") as dram,
            tc.tile_pool(name="const", bufs=1) as constp,
            tc.tile_pool(name="state", bufs=1) as statep,
        ):
            xgT = dram.tile([128, S, 64], BF16)     # [p, t, m*4+b]
            hsT = dram.tile([D, S * B_LOC], BF16)   # [d, 4t+b]

            wh_sb = constp.tile([128, KT, MT, 128], BF16)
            wi_sb = constp.tile([128, KT, MT, 128], BF16)
            ident = constp.tile([128, 128], BF16)
            bi_sb = constp.tile([128, MT], F32)
            epst = constp.tile([128, 1], F32)
            nc.sync.dma_start(wh_sb[:], whp)
            nc.sync.dma_start(wi_sb[:], wip)
            nc.sync.dma_start(ident[:], id_d)
            nc.sync.dma_start(bi_sb[:], bi_d)
            nc.gpsimd.memset(epst[:], EPS)

            # ---------------- Phase AB: LN1 + xg GEMM ----------------
            with (
                tc.tile_pool(name="ab_x", bufs=3) as xp,
                tc.tile_pool(name="ab_ln", bufs=4) as lnp,
                tc.tile_pool(name="ab_zT", bufs=2) as zTp,
                tc.tile_pool(name="ab_ps", bufs=4, space="PSUM") as psp,
                tc.tile_pool(name="ab_stag", bufs=2) as stagp,
            ):
                for c in range(n_chunks):
                    zT = zTp.tile([128, KT, 512], BF16, tag="zT")
                    for q in range(4):
                        xt = xp.tile([128, D], F32, tag="xt")
                        nc.sync.dma_start(xt[:], x_tile_src(xs, c, q))
                        bn6 = lnp.tile([128, 6], F32, tag="bn6")
                        nc.vector.bn_stats(bn6[:], xt[:])
                        mv = lnp.tile([128, 2], F32, tag="mv")
                        nc.vector.bn_aggr(mv[:], bn6[:])
                        sd = lnp.tile([128, 1], F32, tag="sd")
                        nc.scalar.activation(sd[:], mv[:, 1:2], AF.Sqrt, bias=epst[:])
                        rs = lnp.tile([128, 1], F32, tag="rs")
                        nc.vector.reciprocal(rs[:], sd[:])
                        nmr = lnp.tile([128, 1], F32, tag="nmr")
                        nc.vector.tensor_mul(nmr[:], mv[:, 0:1], rs[:])
                        nmrn = lnp.tile([128, 1], F32, tag="nmrn")
                        nc.vector.tensor_scalar_mul(nmrn[:], nmr[:], -1.0)
                        zt = xp.tile([128, D], BF16, tag="zt")
                        nc.scalar.activation(zt[:], xt[:], AF.Identity,
                                             bias=nmrn[:], scale=rs[:])
                        for k in range(KT):
                            nc.sync.dma_start_transpose(
                                zT[:, k, ts(q, 128)], zt[:, ts(k, 128)])
                    stag = stagp.tile([128, TCH, 64], BF16, tag="stag")
                    for m in range(MT):
                        ps = psp.tile([128, 512], F32, tag="ab_ps")
                        for k in range(KT):
                            nc.tensor.matmul(ps[:], wi_sb[:, k, m, :], zT[:, k, :],
                                             start=(k == 0), stop=(k == KT - 1))
                        nc.scalar.activation(stag[:, :, ts(m, 4)], ps[:],
                                             AF.Identity, bias=bi_sb[:, m:m + 1])
                    nc.sync.dma_start(xgT[:, ts(c, TCH), :], stag[:])

            # ---------------- Phase C: LSTM recurrence ----------------
            c2 = statep.tile([128, 2, 16], F32)        # c' = 2c, [parity]
            hst = statep.tile([128, KT, 4 * UNROLL], BF16)  # h.T slots per step
            nc.gpsimd.memset(c2[:], 0.0)
            nc.gpsimd.memset(hst[:], 0.0)

            with (
                tc.tile_pool(name="c_xg", bufs=6) as xgp,
                tc.tile_pool(name="c_psA", bufs=2, space="PSUM") as psA,
                tc.tile_pool(name="c_psB", bufs=2, space="PSUM") as psB,
                tc.tile_pool(name="c_gate", bufs=2) as gp,
            ):
                with tc.For_i(0, S, UNROLL) as t0:
                    for j in range(UNROLL):
                        xg_t = xgp.tile([128, 64], BF16, tag="xg")
                        nc.sync.dma_start(xg_t[:], xgT[:, ds(t0 + j, 1), :])
                        pig = psA.tile([128, 48], F32, tag="pig")
                        pf = psB.tile([128, 16], F32, tag="pf")
                        nc.tensor.matmul(pig[:], ident[:], xg_t[:, 0:48],
                                         start=True, stop=False,
                                         skip_group_check=True)
                        nc.tensor.matmul(pf[:], ident[:], xg_t[:, 48:64],
                                         start=True, stop=False,
                                         skip_group_check=True)
                        jp = (j - 1) % UNROLL
                        for m in range(12):
                            for k in range(KT):
                                nc.tensor.matmul(
                                    pig[:, ts(m, 4)], wh_sb[:, k, m, :],
                                    hst[:, k, ts(jp, 4)],
                                    start=False, stop=(k == KT - 1),
                                    skip_group_check=True)
                        for m in range(12, 16):
                            for k in range(KT):
                                nc.tensor.matmul(
                                    pf[:, ts(m - 12, 4)], wh_sb[:, k, m, :],
                                    hst[:, k, ts(jp, 4)],
                                    start=False, stop=(k == KT - 1),
                                    skip_group_check=True)
                        sig = gp.tile([128, 48], F32, tag="sig")
                        nc.scalar.activation(sig[:], pig[:], AF.Sigmoid)
                        tg2 = gp.tile([128, 16], F32, tag="tg2")
                        nc.vector.tensor_scalar(tg2[:], sig[:, 16:32], 4.0, -2.0,
                                                ALU.mult, ALU.add)
                        t1 = gp.tile([128, 16], F32, tag="t1")
                        nc.vector.tensor_mul(t1[:], sig[:, 0:16], tg2[:])
                        sf = gp.tile([128, 16], F32, tag="sf")
                        nc.scalar.activation(sf[:], pf[:], AF.Sigmoid)
                        t2 = gp.tile([128, 16], F32, tag="t2")
                        nc.vector.tensor_mul(t2[:], sf[:], c2[:, (j + 1) % 2, :])
                        nc.vector.tensor_add(c2[:, j % 2, :], t1[:], t2[:])
                        sc = gp.tile([128, 16], F32, tag="sc")
                        nc.scalar.activation(sc[:], c2[:, j % 2, :], AF.Sigmoid)
                        th = gp.tile([128, 16], F32, tag="th")
                        nc.vector.tensor_scalar(th[:], sc[:], 2.0, -1.0,
                                                ALU.mult, ALU.add)
                        nc.vector.tensor_mul(hst[:, :, ts(j, 4)], sig[:, 32:48],
                                             th[:])
                    for k in range(KT):
                        nc.sync.dma_start(
                            hsT[ts(k, 128), ds(t0 * B_LOC, B_LOC * UNROLL)],
                            hst[:, k, :])

            # ---------------- Phase D: residual + LN2 + MLP ----------------
            w1_sb = constp.tile([128, KT, MT, 128], BF16)
            w2_sb = constp.tile([128, MT, KT, 128], BF16)
            b1_sb = constp.tile([128, MT], F32)
            b2_sb = constp.tile([128, KT], F32)
            nc.sync.dma_start(w1_sb[:], w1p)
            nc.sync.dma_start(w2_sb[:], w2p)
            nc.sync.dma_start(b1_sb[:], b1_d)
            nc.sync.dma_start(b2_sb[:], b2_d)

            with (
                tc.tile_pool(name="d_x", bufs=3) as dxp,
                tc.tile_pool(name="d_x2", bufs=2) as dx2p,
                tc.tile_pool(name="d_ln", bufs=4) as dlnp,
                tc.tile_pool(name="d_zT", bufs=2) as dzTp,
                tc.tile_pool(name="d_u", bufs=2) as dup,
                tc.tile_pool(name="d_ps", bufs=4, space="PSUM") as dpsp,
                tc.tile_pool(name="d_y", bufs=2) as dyp,
            ):
                for c in range(n_chunks):
                    x2 = dx2p.tile([128, 4, D], F32, tag="x2")
                    z2T = dzTp.tile([128, KT, 512], BF16, tag="z2T")
                    for q in range(4):
                        hsq = dxp.tile([128, D], BF16, tag="hsq")
                        for k in range(KT):
                            nc.sync.dma_start_transpose(
                                hsq[:, ts(k, 128)],
                                hsT[ts(k, 128), ts(c * 4 + q, 128)])
                        xt = dxp.tile([128, D], F32, tag="dxt")
                        nc.sync.dma_start(xt[:], x_tile_src(xs, c, q))
                        nc.vector.tensor_add(x2[:, q, :], xt[:], hsq[:])
                        bn6 = dlnp.tile([128, 6], F32, tag="bn6")
                        nc.vector.bn_stats(bn6[:], x2[:, q, :])
                        mv = dlnp.tile([128, 2], F32, tag="mv")
                        nc.vector.bn_aggr(mv[:], bn6[:])
                        sd = dlnp.tile([128, 1], F32, tag="sd")
                        nc.scalar.activation(sd[:], mv[:, 1:2], AF.Sqrt, bias=epst[:])
                        rs = dlnp.tile([128, 1], F32, tag="rs")
                        nc.vector.reciprocal(rs[:], sd[:])
                        nmr = dlnp.tile([128, 1], F32, tag="nmr")
                        nc.vector.tensor_mul(nmr[:], mv[:, 0:1], rs[:])
                        nmrn = dlnp.tile([128, 1], F32, tag="nmrn")
                        nc.vector.tensor_scalar_mul(nmrn[:], nmr[:], -1.0)
                        z2t = dxp.tile([128, D], BF16, tag="z2t")
                        nc.scalar.activation(z2t[:], x2[:, q, :], AF.Identity,
                                             bias=nmrn[:], scale=rs[:])
                        for k in range(KT):
                            nc.sync.dma_start_transpose(
                                z2T[:, k, ts(q, 128)], z2t[:, ts(k, 128)])
                    u = dup.tile([128, MT, 512], BF16, tag="u")
                    for m in range(MT):
                        ps = dpsp.tile([128, 512], F32, tag="d_ps1")
                        for k in range(KT):
                            nc.tensor.matmul(ps[:], w1_sb[:, k, m, :],
                                             z2T[:, k, :],
                                             start=(k == 0), stop=(k == KT - 1))
                        nc.scalar.activation(u[:, m, :], ps[:],
                                             AF.Gelu_apprx_tanh,
                                             bias=b1_sb[:, m:m + 1])
                    yT = dyp.tile([128, KT, 512], BF16, tag="yT")
                    for mo in range(KT):
                        ps2 = dpsp.tile([128, 512], F32, tag="d_ps2")
                        for k in range(MT):
                            nc.tensor.matmul(ps2[:], w2_sb[:, k, mo, :],
                                             u[:, k, :],
                                             start=(k == 0), stop=(k == MT - 1))
                        nc.scalar.activation(yT[:, mo, :], ps2[:], AF.Identity,
                                             bias=b2_sb[:, mo:mo + 1])
                    for q in range(4):
                        yq = dxp.tile([128, D], BF16, tag="yq")
                        for k in range(KT):
                            nc.sync.dma_start_transpose(
                                yq[:, ts(k, 128)], yT[:, k, ts(q, 128)])
                        outq = dxp.tile([128, D], F32, tag="outq")
                        nc.vector.tensor_add(outq[:], x2[:, q, :], yq[:])
                        nc.sync.dma_start(x_tile_src(out, c, q), outq[:])

    nc.compile()
    return nc


_CACHE = {}


def _get_nc(S):
    if S not in _CACHE:
        _CACHE[S] = _build(S)
    return _CACHE[S]


def _prep_weights(ln1_scale, ln1_bias, Wi, Wh, b_lstm, ln2_scale, ln2_bias,
                  W1, b1, W2, b2):
    f32 = np.float32
    bf16 = ml_dtypes.bfloat16
    d = Wi.shape[0]
    # gate permutation [i, g, o, f]; g-gate columns scaled by 2
    perm = np.concatenate([np.arange(0, d), np.arange(2 * d, 3 * d),
                           np.arange(3 * d, 4 * d), np.arange(d, 2 * d)])
    gscale = np.ones(4 * d, f32)
    gscale[d:2 * d] = 2.0

    Wi_f = ((ln1_scale[:, None] * Wi)[:, perm] * gscale[None, :]).astype(f32)
    bi_f = (((b_lstm + ln1_bias @ Wi)[perm]) * gscale).astype(f32)
    Wh_f = (Wh[:, perm] * gscale[None, :]).astype(f32)
    W1_f = (ln2_scale[:, None] * W1).astype(f32)
    b1_f = (b1 + ln2_bias @ W1).astype(f32)

    def pack_kxm(W):  # (K, M) -> (128, K/128, M/128, 128) lhsT tiles
        K, M = W.shape
        return np.ascontiguousarray(
            W.reshape(K // 128, 128, M // 128, 128).transpose(1, 0, 2, 3)
        ).astype(bf16)

    def pack_bias(b):  # (M,) -> (128, M/128): [p, m]
        return np.ascontiguousarray(
            b.reshape(-1, 128).T).astype(f32)

    return {
        "whp": pack_kxm(Wh_f),
        "wip": pack_kxm(Wi_f),
        "w1p": pack_kxm(W1_f),
        "w2p": pack_kxm(W2.astype(f32)),
        "bi": pack_bias(bi_f),
        "b1": pack_bias(b1_f),
        "b2": pack_bias(b2),
        "ident": np.eye(128, dtype=bf16),
    }


def kernel(x, ln1_scale, ln1_bias, Wi, Wh, b_lstm, ln2_scale, ln2_bias,
           W1, b1, W2, b2, _trace=False):
    x = np.asarray(x, np.float32)
    B, S, d = x.shape
    assert d == D and B % N_CORES == 0
    nc = _get_nc(S)
    weights = _prep_weights(
        np.asarray(ln1_scale, np.float32), np.asarray(ln1_bias, np.float32),
        np.asarray(Wi, np.float32), np.asarray(Wh, np.float32),
        np.asarray(b_lstm, np.float32), np.asarray(ln2_scale, np.float32),
        np.asarray(ln2_bias, np.float32), np.asarray(W1, np.float32),
        np.asarray(b1, np.float32), np.asarray(W2, np.float32),
        np.asarray(b2, np.float32))
    bl = B // N_CORES
    in_maps = []
    for c in range(N_CORES):
        m = dict(weights)
        m["xs"] = np.ascontiguousarray(x[c * bl:(c + 1) * bl])
        in_maps.append(m)
    res = run_bass_kernel_spmd(nc, in_maps, core_ids=list(range(N_CORES)),
                               trace=_trace)
    outs = [r["out"] for r in res.results]
    full = np.concatenate(outs, axis=0).astype(np.float32)
    if _trace:
        kernel._last_exec_time_ns = res.exec_time_ns
    return full


# revision 17
# speedup vs baseline: 1.8321x; 1.8321x over previous
"""Trainium2 Bass kernel for nn_ARBlock (LN -> LSTM residual; LN -> MLP residual).

Strategy: pure data-parallel over batch (B=32 -> 4 examples/core on 8 cores,
no collectives).  Per core:
  Phase AB: LN1 + input-gate GEMM  xg.T = Wi'.T @ z.T   (bf16, written to DRAM)
  Phase C : sequential LSTM recurrence over S=2048 steps.  Per step the
            gates land transposed in PSUM: gates.T[m-tile] = sum_k Wh'[k,m].T @ h.T[k],
            with xg injected into PSUM first via an identity matmul so the
            Wh matmuls accumulate on top.  Gate math is sigmoid-only:
            the g-gate columns of Wi'/Wh' are pre-scaled by 2 on the host so
            tanh(g) = 2*sigmoid(2g) - 1, and the cell state is tracked as
            c' = 2c so tanh(c) = 2*sigmoid(c') - 1.
  Phase D : residual + LN2 + MLP (gelu-tanh), residual add, output.

Gate column order is permuted on the host to [f, i, g, o] and the gates land
in three PSUM banks: [f,i] | [g] | [o].  The o-gate matmuls run LAST so the
long cell chain (tanh(g) -> c -> tanh(c)) hides under them; the post-matmul
tail is just sigmoid(o) and one multiply.
"""

import numpy as np
import ml_dtypes

import concourse.bass as bass
import concourse.tile as tile
from concourse import bacc, mybir
from concourse.bass import ts, ds
from concourse.bass_utils import run_bass_kernel_spmd

AF = mybir.ActivationFunctionType
ALU = mybir.AluOpType
F32 = mybir.dt.float32
BF16 = mybir.dt.bfloat16

D = 512
F = 4 * D          # 2048 gate dim
KT = D // 128      # 4 k tiles
MT = F // 128      # 16 m tiles
B_LOC = 4          # batch per core
N_CORES = 8
EPS = 1e-6
UNROLL = 64        # recurrence steps per For_i body
TCH = 128          # recurrence steps per phase-AB/D chunk (512 tokens)


def _build(S):
    """Build the per-core Bass graph.  Returns compiled nc."""
    nc = bacc.Bacc(
        "TRN2",
        target_bir_lowering=False,
        debug=False,
        enable_asserts=False,
        num_devices=N_CORES,
    )
    TOK = B_LOC * S
    n_chunks = S // TCH

    xs = nc.dram_tensor("xs", [B_LOC, S, D], F32, kind="ExternalInput").ap()
    whp = nc.dram_tensor("whp", [128, KT, MT, 128], BF16, kind="ExternalInput").ap()
    wip = nc.dram_tensor("wip", [128, KT, MT, 128], BF16, kind="ExternalInput").ap()
    w1p = nc.dram_tensor("w1p", [128, KT, MT, 128], BF16, kind="ExternalInput").ap()
    w2p = nc.dram_tensor("w2p", [128, MT, KT, 128], BF16, kind="ExternalInput").ap()
    bi_d = nc.dram_tensor("bi", [128, MT], F32, kind="ExternalInput").ap()
    b1_d = nc.dram_tensor("b1", [128, MT], F32, kind="ExternalInput").ap()
    b2_d = nc.dram_tensor("b2", [128, KT], F32, kind="ExternalInput").ap()
    id_d = nc.dram_tensor("ident", [128, 128], BF16, kind="ExternalInput").ap()
    out = nc.dram_tensor("out", [B_LOC, S, D], F32, kind="ExternalOutput").ap()

    def x_tile_src(arr, c, q):
        # 128 tokens (32 t-steps x 4 batch), token index = 4*t + b
        t0 = c * TCH + q * 32
        return arr[:, t0:t0 + 32, :].transpose([1, 0, 2])

    with tile.TileContext(nc) as tc:
        with (
            tc.tile_pool(name="dram", bufs=1, space="DRAM") as dram,
            tc.tile_pool(name="const", bufs=1) as constp,
            tc.tile_pool(name="state", bufs=1) as statep,
        ):
            xgT = dram.tile([128, S, 64], BF16)     # [p, t, m*4+b]
            hsT = dram.tile([D, S * B_LOC], BF16)   # [d, 4t+b]
            hsT_pkc = hsT[:].rearrange("(k p) c -> k p c", p=128).transpose([1, 0, 2])

            wh_sb = constp.tile([128, KT, MT, 128], BF16)
            wi_sb = constp.tile([128, KT, MT, 128], BF16)
            ident = constp.tile([128, 128], BF16)
            bi_sb = constp.tile([128, MT], F32)
            epst = constp.tile([128, 1], F32)
            nc.sync.dma_start(wh_sb[:], whp)
            nc.sync.dma_start(wi_sb[:], wip)
            nc.sync.dma_start(ident[:], id_d)
            nc.sync.dma_start(bi_sb[:], bi_d)
            nc.gpsimd.memset(epst[:], EPS)

            # ---------------- Phase AB: LN1 + xg GEMM ----------------
            with (
                tc.tile_pool(name="ab_x", bufs=3) as xp,
                tc.tile_pool(name="ab_ln", bufs=4) as lnp,
                tc.tile_pool(name="ab_zT", bufs=2) as zTp,
                tc.tile_pool(name="ab_ps", bufs=4, space="PSUM") as psp,
                tc.tile_pool(name="ab_stag", bufs=2) as stagp,
            ):
                for c in range(n_chunks):
                    zT = zTp.tile([128, KT, 512], BF16, tag="zT")
                    for q in range(4):
                        xt = xp.tile([128, D], F32, tag="xt")
                        nc.sync.dma_start(xt[:], x_tile_src(xs, c, q))
                        bn6 = lnp.tile([128, 6], F32, tag="bn6")
                        nc.vector.bn_stats(bn6[:], xt[:])
                        mv = lnp.tile([128, 2], F32, tag="mv")
                        nc.vector.bn_aggr(mv[:], bn6[:])
                        sd = lnp.tile([128, 1], F32, tag="sd")
                        nc.scalar.activation(sd[:], mv[:, 1:2], AF.Sqrt, bias=epst[:])
                        rs = lnp.tile([128, 1], F32, tag="rs")
                        nc.vector.reciprocal(rs[:], sd[:])
                        nmr = lnp.tile([128, 1], F32, tag="nmr")
                        nc.vector.tensor_mul(nmr[:], mv[:, 0:1], rs[:])
                        nmrn = lnp.tile([128, 1], F32, tag="nmrn")
                        nc.vector.tensor_scalar_mul(nmrn[:], nmr[:], -1.0)
                        zt = xp.tile([128, D], BF16, tag="zt")
                        nc.scalar.activation(zt[:], xt[:], AF.Identity,
                                             bias=nmrn[:], scale=rs[:])
                        nc.sync.dma_start_transpose(zT[:, :, ts(q, 128)], zt[:])
                    stag = stagp.tile([128, TCH, 64], BF16, tag="stag")
                    for m in range(MT):
                        ps = psp.tile([128, 512], F32, tag="ab_ps")
                        for k in range(KT):
                            nc.tensor.matmul(ps[:], wi_sb[:, k, m, :], zT[:, k, :],
                                             start=(k == 0), stop=(k == KT - 1))
                        nc.scalar.activation(stag[:, :, ts(m, 4)], ps[:],
                                             AF.Identity, bias=bi_sb[:, m:m + 1])
                    nc.sync.dma_start(xgT[:, ts(c, TCH), :], stag[:])

            # ---------------- Phase C: LSTM recurrence ----------------
            c2 = statep.tile([128, 2, 16], F32)        # cell state, [parity]
            hst = statep.tile([128, KT, 4 * UNROLL], BF16)  # h.T slots per step
            nc.gpsimd.memset(c2[:], 0.0)
            nc.gpsimd.memset(hst[:], 0.0)

            with (
                tc.tile_pool(name="c_xg", bufs=6) as xgp,
                tc.tile_pool(name="c_psA", bufs=2, space="PSUM") as psA,
                tc.tile_pool(name="c_psB", bufs=2, space="PSUM") as psB,
                tc.tile_pool(name="c_psC", bufs=2, space="PSUM") as psC,
                tc.tile_pool(name="c_gate", bufs=2) as gp,
            ):
                with tc.For_i(0, S, UNROLL,
                              hint_engines=(mybir.EngineType.PE,)) as t0:
                    for j in range(UNROLL):
                        xg_t = xgp.tile([128, 64], BF16, tag="xg")
                        nc.sync.dma_start(xg_t[:], xgT[:, ds(t0 + j, 1), :])
                        # banks: A = [f, i] (m 0-7), B = [g] (m 8-11),
                        #        C = [o] (m 12-15); o runs last.
                        pfi = psA.tile([128, 32], F32, tag="pfi")
                        pg = psB.tile([128, 16], F32, tag="pg")
                        po = psC.tile([128, 16], F32, tag="po")
                        nc.tensor.matmul(pfi[:], ident[:], xg_t[:, 0:32],
                                         start=True, stop=False,
                                         skip_group_check=True)
                        nc.tensor.matmul(pg[:], ident[:], xg_t[:, 32:48],
                                         start=True, stop=False,
                                         skip_group_check=True)
                        nc.tensor.matmul(po[:], ident[:], xg_t[:, 48:64],
                                         start=True, stop=False,
                                         skip_group_check=True)
                        jp = (j - 1) % UNROLL

                        def wh_mms(bank, m0, nm):
                            for m in range(m0, m0 + nm):
                                for k in range(KT):
                                    nc.tensor.matmul(
                                        bank[:, ts(m - m0, 4)], wh_sb[:, k, m, :],
                                        hst[:, k, ts(jp, 4)],
                                        start=False, stop=(k == KT - 1),
                                        skip_group_check=True)

                        wh_mms(pfi, 0, 8)
                        sfi = gp.tile([128, 32], F32, tag="sfi")
                        nc.scalar.activation(sfi[:], pfi[:], AF.Sigmoid)
                        t2 = gp.tile([128, 16], F32, tag="t2")
                        nc.vector.tensor_mul(t2[:], sfi[:, 0:16],
                                             c2[:, (j + 1) % 2, :])
                        wh_mms(pg, 8, 4)
                        tg = gp.tile([128, 16], F32, tag="tg")
                        nc.scalar.activation(tg[:], pg[:], AF.Tanh)
                        t1 = gp.tile([128, 16], F32, tag="t1")
                        nc.vector.tensor_mul(t1[:], sfi[:, 16:32], tg[:])
                        nc.vector.tensor_add(c2[:, j % 2, :], t1[:], t2[:])
                        tch = gp.tile([128, 16], F32, tag="tch")
                        nc.scalar.activation(tch[:], c2[:, j % 2, :], AF.Tanh)
                        wh_mms(po, 12, 4)
                        so = gp.tile([128, 16], F32, tag="so")
                        nc.scalar.activation(so[:], po[:], AF.Sigmoid)
                        nc.vector.tensor_mul(hst[:, :, ts(j, 4)], so[:], tch[:])
                    for k in range(KT):
                        nc.sync.dma_start(
                            hsT[ts(k, 128), ds(t0 * B_LOC, B_LOC * UNROLL)],
                            hst[:, k, :])

            # ---------------- Phase D: residual + LN2 + MLP ----------------
            w1_sb = constp.tile([128, KT, MT, 128], BF16)
            w2_sb = constp.tile([128, MT, KT, 128], BF16)
            b1_sb = constp.tile([128, MT], F32)
            b2_sb = constp.tile([128, KT], F32)
            nc.sync.dma_start(w1_sb[:], w1p)
            nc.sync.dma_start(w2_sb[:], w2p)
            nc.sync.dma_start(b1_sb[:], b1_d)
            nc.sync.dma_start(b2_sb[:], b2_d)

            with (
                tc.tile_pool(name="d_x", bufs=3) as dxp,
                tc.tile_pool(name="d_x2", bufs=2) as dx2p,
                tc.tile_pool(name="d_ln", bufs=4) as dlnp,
                tc.tile_pool(name="d_zT", bufs=2) as dzTp,
                tc.tile_pool(name="d_u", bufs=2) as dup,
                tc.tile_pool(name="d_ps", bufs=4, space="PSUM") as dpsp,
                tc.tile_pool(name="d_y", bufs=2) as dyp,
            ):
                for c in range(n_chunks):
                    x2 = dx2p.tile([128, 4, D], F32, tag="x2")
                    z2T = dzTp.tile([128, KT, 512], BF16, tag="z2T")
                    hs_all = dx2p.tile([128, 4, D], BF16, tag="hs_all")
                    for k in range(KT):
                        nc.sync.dma_start_transpose(
                            hs_all[:, :, ts(k, 128)],
                            hsT[ts(k, 128), ts(c, 512)])
                    for q in range(4):
                        xt = dxp.tile([128, D], F32, tag="dxt")
                        nc.sync.dma_start(xt[:], x_tile_src(xs, c, q))
                        nc.vector.tensor_add(x2[:, q, :], xt[:], hs_all[:, q, :])
                        bn6 = dlnp.tile([128, 6], F32, tag="bn6")
                        nc.vector.bn_stats(bn6[:], x2[:, q, :])
                        mv = dlnp.tile([128, 2], F32, tag="mv")
                        nc.vector.bn_aggr(mv[:], bn6[:])
                        sd = dlnp.tile([128, 1], F32, tag="sd")
                        nc.scalar.activation(sd[:], mv[:, 1:2], AF.Sqrt, bias=epst[:])
                        rs = dlnp.tile([128, 1], F32, tag="rs")
                        nc.vector.reciprocal(rs[:], sd[:])
                        nmr = dlnp.tile([128, 1], F32, tag="nmr")
                        nc.vector.tensor_mul(nmr[:], mv[:, 0:1], rs[:])
                        nmrn = dlnp.tile([128, 1], F32, tag="nmrn")
                        nc.vector.tensor_scalar_mul(nmrn[:], nmr[:], -1.0)
                        z2t = dxp.tile([128, D], BF16, tag="z2t")
                        nc.scalar.activation(z2t[:], x2[:, q, :], AF.Identity,
                                             bias=nmrn[:], scale=rs[:])
                        nc.sync.dma_start_transpose(z2T[:, :, ts(q, 128)], z2t[:])
                    u = dup.tile([128, MT, 512], BF16, tag="u")
                    for m in range(MT):
                        ps = dpsp.tile([128, 512], F32, tag="d_ps1")
                        for k in range(KT):
                            nc.tensor.matmul(ps[:], w1_sb[:, k, m, :],
                                             z2T[:, k, :],
                                             start=(k == 0), stop=(k == KT - 1))
                        nc.scalar.activation(u[:, m, :], ps[:],
                                             AF.Gelu_apprx_tanh,
                                             bias=b1_sb[:, m:m + 1])
                    yT = dyp.tile([128, KT, 512], BF16, tag="yT")
                    for mo in range(KT):
                        ps2 = dpsp.tile([128, 512], F32, tag="d_ps2")
                        for k in range(MT):
                            nc.tensor.matmul(ps2[:], w2_sb[:, k, mo, :],
                                             u[:, k, :],
                                             start=(k == 0), stop=(k == MT - 1))
                        nc.scalar.activation(yT[:, mo, :], ps2[:], AF.Identity,
                                             bias=b2_sb[:, mo:mo + 1])
                    yq_all = dx2p.tile([128, 4, D], BF16, tag="yq_all")
                    for k in range(KT):
                        nc.sync.dma_start_transpose(
                            yq_all[:, :, ts(k, 128)], yT[:, k, :])
                    for q in range(4):
                        outq = dxp.tile([128, D], F32, tag="outq")
                        nc.vector.tensor_add(outq[:], x2[:, q, :],
                                             yq_all[:, q, :])
                        nc.sync.dma_start(x_tile_src(out, c, q), outq[:])

    nc.compile()
    return nc


_CACHE = {}


def _get_nc(S):
    if S not in _CACHE:
        _CACHE[S] = _build(S)
    return _CACHE[S]


def _prep_weights(ln1_scale, ln1_bias, Wi, Wh, b_lstm, ln2_scale, ln2_bias,
                  W1, b1, W2, b2):
    f32 = np.float32
    bf16 = ml_dtypes.bfloat16
    d = Wi.shape[0]
    # gate permutation: reference order [i, f, g, o] -> on-chip [f, i, g, o]
    perm = np.concatenate([np.arange(d, 2 * d), np.arange(0, d),
                           np.arange(2 * d, 3 * d), np.arange(3 * d, 4 * d)])

    Wi_f = ((ln1_scale[:, None] * Wi)[:, perm]).astype(f32)
    bi_f = ((b_lstm + ln1_bias @ Wi)[perm]).astype(f32)
    Wh_f = (Wh[:, perm]).astype(f32)
    W1_f = (ln2_scale[:, None] * W1).astype(f32)
    b1_f = (b1 + ln2_bias @ W1).astype(f32)

    def pack_kxm(W):  # (K, M) -> (128, K/128, M/128, 128) lhsT tiles
        K, M = W.shape
        return np.ascontiguousarray(
            W.reshape(K // 128, 128, M // 128, 128).transpose(1, 0, 2, 3)
        ).astype(bf16)

    def pack_bias(b):  # (M,) -> (128, M/128): [p, m]
        return np.ascontiguousarray(
            b.reshape(-1, 128).T).astype(f32)

    return {
        "whp": pack_kxm(Wh_f),
        "wip": pack_kxm(Wi_f),
        "w1p": pack_kxm(W1_f),
        "w2p": pack_kxm(W2.astype(f32)),
        "bi": pack_bias(bi_f),
        "b1": pack_bias(b1_f),
        "b2": pack_bias(b2),
        "ident": np.eye(128, dtype=bf16),
    }


def kernel(x, ln1_scale, ln1_bias, Wi, Wh, b_lstm, ln2_scale, ln2_bias,
           W1, b1, W2, b2, _trace=False):
    x = np.asarray(x, np.float32)
    B, S, d = x.shape
    assert d == D and B % N_CORES == 0
    nc = _get_nc(S)
    weights = _prep_weights(
        np.asarray(ln1_scale, np.float32), np.asarray(ln1_bias, np.float32),
        np.asarray(Wi, np.float32), np.asarray(Wh, np.float32),
        np.asarray(b_lstm, np.float32), np.asarray(ln2_scale, np.float32),
        np.asarray(ln2_bias, np.float32), np.asarray(W1, np.float32),
        np.asarray(b1, np.float32), np.asarray(W2, np.float32),
        np.asarray(b2, np.float32))
    bl = B // N_CORES
    in_maps = []
    for c in range(N_CORES):
        m = dict(weights)
        m["xs"] = np.ascontiguousarray(x[c * bl:(c + 1) * bl])
        in_maps.append(m)
    res = run_bass_kernel_spmd(nc, in_maps, core_ids=list(range(N_CORES)),
                               trace=_trace)
    outs = [r["out"] for r in res.results]
    full = np.concatenate(outs, axis=0).astype(np.float32)
    if _trace:
        kernel._last_exec_time_ns = res.exec_time_ns
    return full


# revision 19
# speedup vs baseline: 1.8516x; 1.0107x over previous
"""Trainium2 Bass kernel for nn_ARBlock (LN -> LSTM residual; LN -> MLP residual).

Strategy: pure data-parallel over batch (B=32 -> 4 examples/core on 8 cores,
no collectives).  Per core:
  Phase AB: LN1 + input-gate GEMM  xg.T = Wi'.T @ z.T   (bf16, written to DRAM)
  Phase C : sequential LSTM recurrence over S=2048 steps.  Per step the
            gates land transposed in PSUM: gates.T[m-tile] = sum_k Wh'[k,m].T @ h.T[k],
            with xg injected into PSUM first via an identity matmul so the
            Wh matmuls accumulate on top.  Gate math is sigmoid-only:
            the g-gate columns of Wi'/Wh' are pre-scaled by 2 on the host so
            tanh(g) = 2*sigmoid(2g) - 1, and the cell state is tracked as
            c' = 2c so tanh(c) = 2*sigmoid(c') - 1.
  Phase D : residual + LN2 + MLP (gelu-tanh), residual add, output.

Gate column order is permuted on the host to [f, i, g, o] and the gates land
in three PSUM banks: [f,i] | [g] | [o].  The o-gate matmuls run LAST so the
long cell chain (tanh(g) -> c -> tanh(c)) hides under them; the post-matmul
tail is just sigmoid(o) and one multiply.
"""

import numpy as np
import ml_dtypes

import concourse.bass as bass
import concourse.tile as tile
from concourse import bacc, mybir
from concourse.bass import ts, ds
from concourse.bass_utils import run_bass_kernel_spmd

AF = mybir.ActivationFunctionType
ALU = mybir.AluOpType
F32 = mybir.dt.float32
BF16 = mybir.dt.bfloat16

D = 512
F = 4 * D          # 2048 gate dim
KT = D // 128      # 4 k tiles
MT = F // 128      # 16 m tiles
B_LOC = 4          # batch per core
N_CORES = 8
EPS = 1e-6
UNROLL = 64        # recurrence steps per For_i body
TCH = 128          # recurrence steps per phase-AB/D chunk (512 tokens)


def _build(S):
    """Build the per-core Bass graph.  Returns compiled nc."""
    nc = bacc.Bacc(
        "TRN2",
        target_bir_lowering=False,
        debug=False,
        enable_asserts=False,
        num_devices=N_CORES,
    )
    TOK = B_LOC * S
    n_chunks = S // TCH

    xs = nc.dram_tensor("xs", [B_LOC, S, D], F32, kind="ExternalInput").ap()
    whp = nc.dram_tensor("whp", [128, KT, MT, 128], BF16, kind="ExternalInput").ap()
    wip = nc.dram_tensor("wip", [128, KT, MT, 128], BF16, kind="ExternalInput").ap()
    w1p = nc.dram_tensor("w1p", [128, KT, MT, 128], BF16, kind="ExternalInput").ap()
    w2p = nc.dram_tensor("w2p", [128, MT, KT, 128], BF16, kind="ExternalInput").ap()
    bi_d = nc.dram_tensor("bi", [128, MT], F32, kind="ExternalInput").ap()
    b1_d = nc.dram_tensor("b1", [128, MT], F32, kind="ExternalInput").ap()
    b2_d = nc.dram_tensor("b2", [128, KT], F32, kind="ExternalInput").ap()
    id_d = nc.dram_tensor("ident", [128, 128], BF16, kind="ExternalInput").ap()
    out = nc.dram_tensor("out", [B_LOC, S, D], F32, kind="ExternalOutput").ap()

    def x_tile_src(arr, c, q):
        # 128 tokens (32 t-steps x 4 batch), token index = 4*t + b
        t0 = c * TCH + q * 32
        return arr[:, t0:t0 + 32, :].transpose([1, 0, 2])

    with tile.TileContext(nc) as tc:
        with (
            tc.tile_pool(name="dram", bufs=1, space="DRAM") as dram,
            tc.tile_pool(name="const", bufs=1) as constp,
            tc.tile_pool(name="state", bufs=1) as statep,
        ):
            xgT = dram.tile([128, S, 64], BF16)     # [p, t, m*4+b]
            hsT = dram.tile([D, S * B_LOC], BF16)   # [d, 4t+b]
            hsT_pkc = hsT[:].rearrange("(k p) c -> k p c", p=128).transpose([1, 0, 2])

            wh_sb = constp.tile([128, KT, MT, 128], BF16)
            wi_sb = constp.tile([128, KT, MT, 128], BF16)
            ident = constp.tile([128, 128], BF16)
            bi_sb = constp.tile([128, MT], F32)
            epst = constp.tile([128, 1], F32)
            nc.sync.dma_start(wh_sb[:], whp)
            nc.sync.dma_start(wi_sb[:], wip)
            nc.sync.dma_start(ident[:], id_d)
            nc.sync.dma_start(bi_sb[:], bi_d)
            nc.gpsimd.memset(epst[:], EPS)

            # ---------------- Phase AB: LN1 + xg GEMM ----------------
            with (
                tc.tile_pool(name="ab_x", bufs=3) as xp,
                tc.tile_pool(name="ab_ln", bufs=4) as lnp,
                tc.tile_pool(name="ab_zT", bufs=2) as zTp,
                tc.tile_pool(name="ab_ps", bufs=4, space="PSUM") as psp,
                tc.tile_pool(name="ab_stag", bufs=2) as stagp,
            ):
                for c in range(n_chunks):
                    zT = zTp.tile([128, KT, 512], BF16, tag="zT")
                    for q in range(4):
                        xt = xp.tile([128, D], F32, tag="xt")
                        nc.sync.dma_start(xt[:], x_tile_src(xs, c, q))
                        bn6 = lnp.tile([128, 6], F32, tag="bn6")
                        nc.vector.bn_stats(bn6[:], xt[:])
                        mv = lnp.tile([128, 2], F32, tag="mv")
                        nc.vector.bn_aggr(mv[:], bn6[:])
                        sd = lnp.tile([128, 1], F32, tag="sd")
                        nc.scalar.activation(sd[:], mv[:, 1:2], AF.Sqrt, bias=epst[:])
                        rs = lnp.tile([128, 1], F32, tag="rs")
                        nc.vector.reciprocal(rs[:], sd[:])
                        nmr = lnp.tile([128, 1], F32, tag="nmr")
                        nc.vector.tensor_mul(nmr[:], mv[:, 0:1], rs[:])
                        nmrn = lnp.tile([128, 1], F32, tag="nmrn")
                        nc.vector.tensor_scalar_mul(nmrn[:], nmr[:], -1.0)
                        zt = xp.tile([128, D], BF16, tag="zt")
                        nc.scalar.activation(zt[:], xt[:], AF.Identity,
                                             bias=nmrn[:], scale=rs[:])
                        nc.sync.dma_start_transpose(zT[:, :, ts(q, 128)], zt[:])
                    stag = stagp.tile([128, TCH, 64], BF16, tag="stag")
                    for m in range(MT):
                        ps = psp.tile([128, 512], F32, tag="ab_ps")
                        for k in range(KT):
                            nc.tensor.matmul(ps[:], wi_sb[:, k, m, :], zT[:, k, :],
                                             start=(k == 0), stop=(k == KT - 1))
                        nc.scalar.activation(stag[:, :, ts(m, 4)], ps[:],
                                             AF.Identity, bias=bi_sb[:, m:m + 1])
                    nc.sync.dma_start(xgT[:, ts(c, TCH), :], stag[:])

            # ---------------- Phase C: LSTM recurrence ----------------
            # ctg[par][0:16] = cell state written by steps of parity par;
            # ctg[par][16:32] = tanh(g) written there by the NEXT step so a
            # single wide multiply computes [f*c_prev | i*tanh_g].
            ctg = statep.tile([128, 2, 32], F32)
            hst = statep.tile([128, KT, 4 * UNROLL], BF16)  # h.T slots per step
            nc.gpsimd.memset(ctg[:], 0.0)
            nc.gpsimd.memset(hst[:], 0.0)

            with (
                tc.tile_pool(name="c_xg", bufs=8) as xgp,
                tc.tile_pool(name="c_psA", bufs=3, space="PSUM") as psA,
                tc.tile_pool(name="c_psB", bufs=3, space="PSUM") as psB,
                tc.tile_pool(name="c_psC", bufs=2, space="PSUM") as psC,
                tc.tile_pool(name="c_gate", bufs=6) as gp,
            ):
                with tc.For_i(0, S, UNROLL,
                              hint_engines=(mybir.EngineType.PE,)) as t0:
                    for j in range(UNROLL):
                        xg_t = xgp.tile([128, 64], BF16, tag="xg")
                        nc.sync.dma_start(xg_t[:], xgT[:, ds(t0 + j, 1), :])
                        # banks: A = [f, i] (m 0-7), B = [g] (m 8-11),
                        #        C = [o] (m 12-15); o runs last.
                        pfi = psA.tile([128, 32], F32, tag="pfi")
                        pg = psB.tile([128, 16], F32, tag="pg")
                        po = psC.tile([128, 16], F32, tag="po")
                        nc.tensor.matmul(pfi[:], ident[:], xg_t[:, 0:32],
                                         start=True, stop=False,
                                         skip_group_check=True)
                        nc.tensor.matmul(pg[:], ident[:], xg_t[:, 32:48],
                                         start=True, stop=False,
                                         skip_group_check=True)
                        nc.tensor.matmul(po[:], ident[:], xg_t[:, 48:64],
                                         start=True, stop=False,
                                         skip_group_check=True)
                        jp = (j - 1) % UNROLL

                        def wh_mms(bank, m0, nm):
                            for m in range(m0, m0 + nm):
                                for k in range(KT):
                                    nc.tensor.matmul(
                                        bank[:, ts(m - m0, 4)], wh_sb[:, k, m, :],
                                        hst[:, k, ts(jp, 4)],
                                        start=False, stop=(k == KT - 1),
                                        skip_group_check=True)

                        pv = (j + 1) % 2
                        cur = j % 2
                        wh_mms(pfi, 0, 8)
                        sfi = gp.tile([128, 32], F32, tag="sfi")
                        nc.scalar.activation(sfi[:], pfi[:], AF.Sigmoid)
                        wh_mms(pg, 8, 4)
                        # tanh(g) lands next to the previous cell state
                        nc.scalar.activation(ctg[:, pv, 16:32], pg[:], AF.Tanh)
                        t12 = gp.tile([128, 32], F32, tag="t12")
                        nc.vector.tensor_mul(t12[:], sfi[:], ctg[:, pv, :])
                        nc.vector.tensor_add(ctg[:, cur, 0:16], t12[:, 0:16],
                                             t12[:, 16:32])
                        tch = gp.tile([128, 16], F32, tag="tch")
                        nc.scalar.activation(tch[:], ctg[:, cur, 0:16], AF.Tanh)
                        wh_mms(po, 12, 4)
                        so = gp.tile([128, 16], F32, tag="so")
                        nc.scalar.activation(so[:], po[:], AF.Sigmoid)
                        nc.vector.tensor_mul(hst[:, :, ts(j, 4)], so[:], tch[:])
                    for k in range(KT):
                        nc.sync.dma_start(
                            hsT[ts(k, 128), ds(t0 * B_LOC, B_LOC * UNROLL)],
                            hst[:, k, :])

            # ---------------- Phase D: residual + LN2 + MLP ----------------
            w1_sb = constp.tile([128, KT, MT, 128], BF16)
            w2_sb = constp.tile([128, MT, KT, 128], BF16)
            b1_sb = constp.tile([128, MT], F32)
            b2_sb = constp.tile([128, KT], F32)
            nc.sync.dma_start(w1_sb[:], w1p)
            nc.sync.dma_start(w2_sb[:], w2p)
            nc.sync.dma_start(b1_sb[:], b1_d)
            nc.sync.dma_start(b2_sb[:], b2_d)

            with (
                tc.tile_pool(name="d_x", bufs=3) as dxp,
                tc.tile_pool(name="d_x2", bufs=2) as dx2p,
                tc.tile_pool(name="d_ln", bufs=4) as dlnp,
                tc.tile_pool(name="d_zT", bufs=2) as dzTp,
                tc.tile_pool(name="d_u", bufs=2) as dup,
                tc.tile_pool(name="d_ps", bufs=4, space="PSUM") as dpsp,
                tc.tile_pool(name="d_y", bufs=2) as dyp,
            ):
                for c in range(n_chunks):
                    x2 = dx2p.tile([128, 4, D], F32, tag="x2")
                    z2T = dzTp.tile([128, KT, 512], BF16, tag="z2T")
                    hs_all = dx2p.tile([128, 4, D], BF16, tag="hs_all")
                    for k in range(KT):
                        nc.sync.dma_start_transpose(
                            hs_all[:, :, ts(k, 128)],
                            hsT[ts(k, 128), ts(c, 512)])
                    for q in range(4):
                        xt = dxp.tile([128, D], F32, tag="dxt")
                        nc.sync.dma_start(xt[:], x_tile_src(xs, c, q))
                        nc.vector.tensor_add(x2[:, q, :], xt[:], hs_all[:, q, :])
                        bn6 = dlnp.tile([128, 6], F32, tag="bn6")
                        nc.vector.bn_stats(bn6[:], x2[:, q, :])
                        mv = dlnp.tile([128, 2], F32, tag="mv")
                        nc.vector.bn_aggr(mv[:], bn6[:])
                        sd = dlnp.tile([128, 1], F32, tag="sd")
                        nc.scalar.activation(sd[:], mv[:, 1:2], AF.Sqrt, bias=epst[:])
                        rs = dlnp.tile([128, 1], F32, tag="rs")
                        nc.vector.reciprocal(rs[:], sd[:])
                        nmr = dlnp.tile([128, 1], F32, tag="nmr")
                        nc.vector.tensor_mul(nmr[:], mv[:, 0:1], rs[:])
                        nmrn = dlnp.tile([128, 1], F32, tag="nmrn")
                        nc.vector.tensor_scalar_mul(nmrn[:], nmr[:], -1.0)
                        z2t = dxp.tile([128, D], BF16, tag="z2t")
                        nc.scalar.activation(z2t[:], x2[:, q, :], AF.Identity,
                                             bias=nmrn[:], scale=rs[:])
                        nc.sync.dma_start_transpose(z2T[:, :, ts(q, 128)], z2t[:])
                    u = dup.tile([128, MT, 512], BF16, tag="u")
                    for m in range(MT):
                        ps = dpsp.tile([128, 512], F32, tag="d_ps1")
                        for k in range(KT):
                            nc.tensor.matmul(ps[:], w1_sb[:, k, m, :],
                                             z2T[:, k, :],
                                             start=(k == 0), stop=(k == KT - 1))
                        nc.scalar.activation(u[:, m, :], ps[:],
                                             AF.Gelu_apprx_tanh,
                                             bias=b1_sb[:, m:m + 1])
                    yT = dyp.tile([128, KT, 512], BF16, tag="yT")
                    for mo in range(KT):
                        ps2 = dpsp.tile([128, 512], F32, tag="d_ps2")
                        for k in range(MT):
                            nc.tensor.matmul(ps2[:], w2_sb[:, k, mo, :],
                                             u[:, k, :],
                                             start=(k == 0), stop=(k == MT - 1))
                        nc.scalar.activation(yT[:, mo, :], ps2[:], AF.Identity,
                                             bias=b2_sb[:, mo:mo + 1])
                    yq_all = dx2p.tile([128, 4, D], BF16, tag="yq_all")
                    for k in range(KT):
                        nc.sync.dma_start_transpose(
                            yq_all[:, :, ts(k, 128)], yT[:, k, :])
                    for q in range(4):
                        outq = dxp.tile([128, D], F32, tag="outq")
                        nc.vector.tensor_add(outq[:], x2[:, q, :],
                                             yq_all[:, q, :])
                        nc.sync.dma_start(x_tile_src(out, c, q), outq[:])

    nc.compile()
    return nc


_CACHE = {}


def _get_nc(S):
    if S not in _CACHE:
        _CACHE[S] = _build(S)
    return _CACHE[S]


def _prep_weights(ln1_scale, ln1_bias, Wi, Wh, b_lstm, ln2_scale, ln2_bias,
                  W1, b1, W2, b2):
    f32 = np.float32
    bf16 = ml_dtypes.bfloat16
    d = Wi.shape[0]
    # gate permutation: reference order [i, f, g, o] -> on-chip [f, i, g, o]
    perm = np.concatenate([np.arange(d, 2 * d), np.arange(0, d),
                           np.arange(2 * d, 3 * d), np.arange(3 * d, 4 * d)])

    Wi_f = ((ln1_scale[:, None] * Wi)[:, perm]).astype(f32)
    bi_f = ((b_lstm + ln1_bias @ Wi)[perm]).astype(f32)
    Wh_f = (Wh[:, perm]).astype(f32)
    W1_f = (ln2_scale[:, None] * W1).astype(f32)
    b1_f = (b1 + ln2_bias @ W1).astype(f32)

    def pack_kxm(W):  # (K, M) -> (128, K/128, M/128, 128) lhsT tiles
        K, M = W.shape
        return np.ascontiguousarray(
            W.reshape(K // 128, 128, M // 128, 128).transpose(1, 0, 2, 3)
        ).astype(bf16)

    def pack_bias(b):  # (M,) -> (128, M/128): [p, m]
        return np.ascontiguousarray(
            b.reshape(-1, 128).T).astype(f32)

    return {
        "whp": pack_kxm(Wh_f),
        "wip": pack_kxm(Wi_f),
        "w1p": pack_kxm(W1_f),
        "w2p": pack_kxm(W2.astype(f32)),
        "bi": pack_bias(bi_f),
        "b1": pack_bias(b1_f),
        "b2": pack_bias(b2),
        "ident": np.eye(128, dtype=bf16),
    }


def kernel(x, ln1_scale, ln1_bias, Wi, Wh, b_lstm, ln2_scale, ln2_bias,
           W1, b1, W2, b2, _trace=False):
    x = np.asarray(x, np.float32)
    B, S, d = x.shape
    assert d == D and B % N_CORES == 0
    nc = _get_nc(S)
    weights = _prep_weights(
        np.asarray(ln1_scale, np.float32), np.asarray(ln1_bias, np.float32),
        np.asarray(Wi, np.float32), np.asarray(Wh, np.float32),
        np.asarray(b_lstm, np.float32), np.asarray(ln2_scale, np.float32),
        np.asarray(ln2_bias, np.float32), np.asarray(W1, np.float32),
        np.asarray(b1, np.float32), np.asarray(W2, np.float32),
        np.asarray(b2, np.float32))
    bl = B // N_CORES
    in_maps = []
    for c in range(N_CORES):
        m = dict(weights)
        m["xs"] = np.ascontiguousarray(x[c * bl:(c + 1) * bl])
        in_maps.append(m)
    res = run_bass_kernel_spmd(nc, in_maps, core_ids=list(range(N_CORES)),
                               trace=_trace)
    outs = [r["out"] for r in res.results]
    full = np.concatenate(outs, axis=0).astype(np.float32)
    if _trace:
        kernel._last_exec_time_ns = res.exec_time_ns
    return full


# revision 25
# speedup vs baseline: 1.9521x; 1.0543x over previous
"""Trainium2 Bass kernel for nn_ARBlock (LN -> LSTM residual; LN -> MLP residual).

Strategy: pure data-parallel over batch (B=32 -> 4 examples/core on 8 cores,
no collectives).  Per core:
  Phase AB: LN1 + input-gate GEMM  xg.T = Wi'.T @ z.T   (bf16, written to DRAM)
  Phase C : sequential LSTM recurrence over S=2048 steps.  Per step the
            gates land transposed in PSUM: gates.T[m-tile] = sum_k Wh'[k,m].T @ h.T[k],
            with xg injected into PSUM first via an identity matmul so the
            Wh matmuls accumulate on top.  Gate math is sigmoid-only:
            the g-gate columns of Wi'/Wh' are pre-scaled by 2 on the host so
            tanh(g) = 2*sigmoid(2g) - 1, and the cell state is tracked as
            c' = 2c so tanh(c) = 2*sigmoid(c') - 1.
  Phase D : residual + LN2 + MLP (gelu-tanh), residual add, output.

Gate column order is permuted on the host to [f, i, g, o] and the gates land
in three PSUM banks: [f,i] | [g] | [o].  The o-gate matmuls run LAST so the
long cell chain (tanh(g) -> c -> tanh(c)) hides under them; the post-matmul
tail is just sigmoid(o) and one multiply.
"""

import numpy as np
import ml_dtypes

import concourse.bass as bass
import concourse.tile as tile
from concourse import bacc, mybir
from concourse.bass import ts, ds
from concourse.bass_utils import run_bass_kernel_spmd

AF = mybir.ActivationFunctionType
ALU = mybir.AluOpType
F32 = mybir.dt.float32
BF16 = mybir.dt.bfloat16

D = 512
F = 4 * D          # 2048 gate dim
KT = D // 128      # 4 k tiles
MT = F // 128      # 16 m tiles
B_LOC = 4          # batch per core
N_CORES = 8
EPS = 1e-6
UNROLL = 64        # recurrence steps per For_i body
TCH = 128          # recurrence steps per phase-AB/D chunk (512 tokens)


def _build(S):
    """Build the per-core Bass graph.  Returns compiled nc."""
    nc = bacc.Bacc(
        "TRN2",
        target_bir_lowering=False,
        debug=False,
        enable_asserts=False,
        num_devices=N_CORES,
    )
    TOK = B_LOC * S
    n_chunks = S // TCH

    xs = nc.dram_tensor("xs", [B_LOC, S, D], F32, kind="ExternalInput").ap()
    whp = nc.dram_tensor("whp", [128, KT, MT, 128], BF16, kind="ExternalInput").ap()
    wip = nc.dram_tensor("wip", [128, KT, MT, 128], BF16, kind="ExternalInput").ap()
    w1p = nc.dram_tensor("w1p", [128, KT, MT, 128], BF16, kind="ExternalInput").ap()
    w2p = nc.dram_tensor("w2p", [128, MT, KT, 128], BF16, kind="ExternalInput").ap()
    bi_d = nc.dram_tensor("bi", [128, MT], F32, kind="ExternalInput").ap()
    b1_d = nc.dram_tensor("b1", [128, MT], F32, kind="ExternalInput").ap()
    b2_d = nc.dram_tensor("b2", [128, KT], F32, kind="ExternalInput").ap()
    id_d = nc.dram_tensor("ident", [128, 128], BF16, kind="ExternalInput").ap()
    out = nc.dram_tensor("out", [B_LOC, S, D], F32, kind="ExternalOutput").ap()

    def x_tile_src(arr, c, q):
        # 128 tokens (32 t-steps x 4 batch), token index = 4*t + b
        t0 = c * TCH + q * 32
        return arr[:, t0:t0 + 32, :].transpose([1, 0, 2])

    with tile.TileContext(nc) as tc:
        with (
            tc.tile_pool(name="dram", bufs=1, space="DRAM") as dram,
            tc.tile_pool(name="const", bufs=1) as constp,
            tc.tile_pool(name="state", bufs=1) as statep,
        ):
            # per-chunk scratch tensors -> precise (tile-level) dependency
            # tracking, so phase D chunk c becomes schedulable as soon as
            # the recurrence finishes chunk c and fills PE bubbles.
            xgTs = [dram.tile([128, TCH, 64], BF16, name=f"xgT{c}",
                              tag=f"xgT{c}") for c in range(n_chunks)]
            hsTs = [dram.tile([D, TCH * B_LOC], BF16, name=f"hsT{c}",
                              tag=f"hsT{c}") for c in range(n_chunks)]

            wh_sb = constp.tile([128, KT, MT, 128], BF16)
            wi_sb = constp.tile([128, KT, MT, 128], BF16)
            ident = constp.tile([128, 128], BF16)
            bi_sb = constp.tile([128, MT], F32)
            epst = constp.tile([128, 1], F32)
            nc.sync.dma_start(wh_sb[:], whp)
            nc.sync.dma_start(wi_sb[:], wip)
            nc.sync.dma_start(ident[:], id_d)
            nc.sync.dma_start(bi_sb[:], bi_d)
            nc.gpsimd.memset(epst[:], EPS)

            # ---------------- Phase AB: LN1 + xg GEMM ----------------
            with (
                tc.tile_pool(name="ab_x", bufs=3) as xp,
                tc.tile_pool(name="ab_ln", bufs=4) as lnp,
                tc.tile_pool(name="ab_zT", bufs=2) as zTp,
                tc.tile_pool(name="ab_ps", bufs=4, space="PSUM") as psp,
                tc.tile_pool(name="ab_stag", bufs=2) as stagp,
            ):
                for c in range(n_chunks):
                    zT = zTp.tile([128, KT, 512], BF16, tag="zT")
                    for q in range(4):
                        xt = xp.tile([128, D], F32, tag="xt")
                        nc.sync.dma_start(xt[:], x_tile_src(xs, c, q))
                        bn6 = lnp.tile([128, 6], F32, tag="bn6")
                        nc.vector.bn_stats(bn6[:], xt[:])
                        mv = lnp.tile([128, 2], F32, tag="mv")
                        nc.vector.bn_aggr(mv[:], bn6[:])
                        sd = lnp.tile([128, 1], F32, tag="sd")
                        nc.scalar.activation(sd[:], mv[:, 1:2], AF.Sqrt, bias=epst[:])
                        rs = lnp.tile([128, 1], F32, tag="rs")
                        nc.vector.reciprocal(rs[:], sd[:])
                        nmr = lnp.tile([128, 1], F32, tag="nmr")
                        nc.vector.tensor_mul(nmr[:], mv[:, 0:1], rs[:])
                        nmrn = lnp.tile([128, 1], F32, tag="nmrn")
                        nc.vector.tensor_scalar_mul(nmrn[:], nmr[:], -1.0)
                        zt = xp.tile([128, D], BF16, tag="zt")
                        nc.scalar.activation(zt[:], xt[:], AF.Identity,
                                             bias=nmrn[:], scale=rs[:])
                        nc.sync.dma_start_transpose(zT[:, :, ts(q, 128)], zt[:])
                    stag = stagp.tile([128, TCH, 64], BF16, tag="stag")
                    for m in range(MT):
                        ps = psp.tile([128, 512], F32, tag="ab_ps")
                        for k in range(KT):
                            nc.tensor.matmul(ps[:], wi_sb[:, k, m, :], zT[:, k, :],
                                             start=(k == 0), stop=(k == KT - 1))
                        nc.scalar.activation(stag[:, :, ts(m, 4)], ps[:],
                                             AF.Identity, bias=bi_sb[:, m:m + 1])
                    nc.sync.dma_start(xgTs[c][:], stag[:])

            # ---------------- Phase C: LSTM recurrence ----------------
            # ctg[par][0:16] = cell state written by steps of parity par;
            # ctg[par][16:32] = tanh(g) written there by the NEXT step so a
            # single wide multiply computes [f*c_prev | i*tanh_g].
            ctg = statep.tile([128, 2, 32], F32)
            hst = statep.tile([128, KT, 4 * UNROLL], BF16)  # h.T slots per step
            nc.gpsimd.memset(ctg[:], 0.0)
            nc.gpsimd.memset(hst[:], 0.0)

            with (
                tc.tile_pool(name="c_xg", bufs=8) as xgp,
                tc.tile_pool(name="c_psA", bufs=3, space="PSUM") as psA,
                tc.tile_pool(name="c_psB", bufs=3, space="PSUM") as psB,
                tc.tile_pool(name="c_psC", bufs=2, space="PSUM") as psC,
                tc.tile_pool(name="c_gate", bufs=6) as gp,
            ):
                for t in range(S):
                    j = t % UNROLL
                    if True:
                        xg_t = xgp.tile([128, 64], BF16, tag="xg")
                        nc.sync.dma_start(xg_t[:],
                                          xgTs[t // TCH][:, t % TCH, :])
                        # banks: A = [f, i] (m 0-7), B = [g] (m 8-11),
                        #        C = [o] (m 12-15); o runs last.
                        pfi = psA.tile([128, 32], F32, tag="pfi")
                        pg = psB.tile([128, 16], F32, tag="pg")
                        po = psC.tile([128, 16], F32, tag="po")
                        nc.tensor.matmul(pfi[:], ident[:], xg_t[:, 0:32],
                                         start=True, stop=False,
                                         skip_group_check=True)
                        nc.tensor.matmul(pg[:], ident[:], xg_t[:, 32:48],
                                         start=True, stop=False,
                                         skip_group_check=True)
                        nc.tensor.matmul(po[:], ident[:], xg_t[:, 48:64],
                                         start=True, stop=False,
                                         skip_group_check=True)
                        jp = (j - 1) % UNROLL

                        def wh_mms(bank, m0, nm):
                            for m in range(m0, m0 + nm):
                                for k in range(KT):
                                    nc.tensor.matmul(
                                        bank[:, ts(m - m0, 4)], wh_sb[:, k, m, :],
                                        hst[:, k, ts(jp, 4)],
                                        start=False, stop=(k == KT - 1),
                                        skip_group_check=True)

                        pv = (j + 1) % 2
                        cur = j % 2
                        wh_mms(pfi, 0, 8)
                        sfi = gp.tile([128, 32], F32, tag="sfi")
                        nc.scalar.activation(sfi[:], pfi[:], AF.Sigmoid)
                        wh_mms(pg, 8, 4)
                        # tanh(g) lands next to the previous cell state
                        nc.scalar.activation(ctg[:, pv, 16:32], pg[:], AF.Tanh)
                        t12 = gp.tile([128, 32], F32, tag="t12")
                        nc.vector.tensor_mul(t12[:], sfi[:], ctg[:, pv, :])
                        nc.vector.tensor_add(ctg[:, cur, 0:16], t12[:, 0:16],
                                             t12[:, 16:32])
                        tch = gp.tile([128, 16], F32, tag="tch")
                        nc.scalar.activation(tch[:], ctg[:, cur, 0:16], AF.Tanh)
                        wh_mms(po, 12, 4)
                        so = gp.tile([128, 16], F32, tag="so")
                        nc.scalar.activation(so[:], po[:], AF.Sigmoid)
                        nc.vector.tensor_mul(hst[:, :, ts(j, 4)], so[:], tch[:])
                    if j == UNROLL - 1:
                        tl0 = (t - j) % TCH
                        for k in range(KT):
                            nc.sync.dma_start(
                                hsTs[t // TCH][ts(k, 128),
                                               tl0 * B_LOC:
                                               (tl0 + UNROLL) * B_LOC],
                                hst[:, k, :])

            # ---------------- Phase D: residual + LN2 + MLP ----------------
            w1_sb = constp.tile([128, KT, MT, 128], BF16)
            w2_sb = constp.tile([128, MT, KT, 128], BF16)
            b1_sb = constp.tile([128, MT], F32)
            b2_sb = constp.tile([128, KT], F32)
            nc.sync.dma_start(w1_sb[:], w1p)
            nc.sync.dma_start(w2_sb[:], w2p)
            nc.sync.dma_start(b1_sb[:], b1_d)
            nc.sync.dma_start(b2_sb[:], b2_d)

            with (
                tc.tile_pool(name="d_x", bufs=3) as dxp,
                tc.tile_pool(name="d_x2", bufs=2) as dx2p,
                tc.tile_pool(name="d_ln", bufs=4) as dlnp,
                tc.tile_pool(name="d_zT", bufs=2) as dzTp,
                tc.tile_pool(name="d_u", bufs=2) as dup,
                tc.tile_pool(name="d_ps", bufs=4, space="PSUM") as dpsp,
                tc.tile_pool(name="d_y", bufs=2) as dyp,
            ):
                for c in range(n_chunks):
                    x2 = dx2p.tile([128, 4, D], F32, tag="x2")
                    z2T = dzTp.tile([128, KT, 512], BF16, tag="z2T")
                    hs_all = dx2p.tile([128, 4, D], BF16, tag="hs_all")
                    for k in range(KT):
                        nc.sync.dma_start_transpose(
                            hs_all[:, :, ts(k, 128)],
                            hsTs[c][ts(k, 128), :])
                    for q in range(4):
                        xt = dxp.tile([128, D], F32, tag="dxt")
                        nc.sync.dma_start(xt[:], x_tile_src(xs, c, q))
                        nc.vector.tensor_add(x2[:, q, :], xt[:], hs_all[:, q, :])
                        bn6 = dlnp.tile([128, 6], F32, tag="bn6")
                        nc.vector.bn_stats(bn6[:], x2[:, q, :])
                        mv = dlnp.tile([128, 2], F32, tag="mv")
                        nc.vector.bn_aggr(mv[:], bn6[:])
                        sd = dlnp.tile([128, 1], F32, tag="sd")
                        nc.scalar.activation(sd[:], mv[:, 1:2], AF.Sqrt, bias=epst[:])
                        rs = dlnp.tile([128, 1], F32, tag="rs")
                        nc.vector.reciprocal(rs[:], sd[:])
                        nmr = dlnp.tile([128, 1], F32, tag="nmr")
                        nc.vector.tensor_mul(nmr[:], mv[:, 0:1], rs[:])
                        nmrn = dlnp.tile([128, 1], F32, tag="nmrn")
                        nc.vector.tensor_scalar_mul(nmrn[:], nmr[:], -1.0)
                        z2t = dxp.tile([128, D], BF16, tag="z2t")
                        nc.scalar.activation(z2t[:], x2[:, q, :], AF.Identity,
                                             bias=nmrn[:], scale=rs[:])
                        nc.sync.dma_start_transpose(z2T[:, :, ts(q, 128)], z2t[:])
                    u = dup.tile([128, MT, 512], BF16, tag="u")
                    for m in range(MT):
                        ps = dpsp.tile([128, 512], F32, tag="d_ps1")
                        for k in range(KT):
                            nc.tensor.matmul(ps[:], w1_sb[:, k, m, :],
                                             z2T[:, k, :],
                                             start=(k == 0), stop=(k == KT - 1))
                        nc.scalar.activation(u[:, m, :], ps[:],
                                             AF.Gelu_apprx_tanh,
                                             bias=b1_sb[:, m:m + 1])
                    yT = dyp.tile([128, KT, 512], BF16, tag="yT")
                    for mo in range(KT):
                        ps2 = dpsp.tile([128, 512], F32, tag="d_ps2")
                        for k in range(MT):
                            nc.tensor.matmul(ps2[:], w2_sb[:, k, mo, :],
                                             u[:, k, :],
                                             start=(k == 0), stop=(k == MT - 1))
                        nc.scalar.activation(yT[:, mo, :], ps2[:], AF.Identity,
                                             bias=b2_sb[:, mo:mo + 1])
                    yq_all = dx2p.tile([128, 4, D], BF16, tag="yq_all")
                    for k in range(KT):
                        nc.sync.dma_start_transpose(
                            yq_all[:, :, ts(k, 128)], yT[:, k, :])
                    for q in range(4):
                        outq = dxp.tile([128, D], F32, tag="outq")
                        nc.vector.tensor_add(outq[:], x2[:, q, :],
                                             yq_all[:, q, :])
                        nc.sync.dma_start(x_tile_src(out, c, q), outq[:])

    nc.compile()
    return nc


_CACHE = {}


def _get_nc(S):
    if S not in _CACHE:
        _CACHE[S] = _build(S)
    return _CACHE[S]


def _prep_weights(ln1_scale, ln1_bias, Wi, Wh, b_lstm, ln2_scale, ln2_bias,
                  W1, b1, W2, b2):
    f32 = np.float32
    bf16 = ml_dtypes.bfloat16
    d = Wi.shape[0]
    # gate permutation: reference order [i, f, g, o] -> on-chip [f, i, g, o]
    perm = np.concatenate([np.arange(d, 2 * d), np.arange(0, d),
                           np.arange(2 * d, 3 * d), np.arange(3 * d, 4 * d)])

    Wi_f = ((ln1_scale[:, None] * Wi)[:, perm]).astype(f32)
    bi_f = ((b_lstm + ln1_bias @ Wi)[perm]).astype(f32)
    Wh_f = (Wh[:, perm]).astype(f32)
    W1_f = (ln2_scale[:, None] * W1).astype(f32)
    b1_f = (b1 + ln2_bias @ W1).astype(f32)

    def pack_kxm(W):  # (K, M) -> (128, K/128, M/128, 128) lhsT tiles
        K, M = W.shape
        return np.ascontiguousarray(
            W.reshape(K // 128, 128, M // 128, 128).transpose(1, 0, 2, 3)
        ).astype(bf16)

    def pack_bias(b):  # (M,) -> (128, M/128): [p, m]
        return np.ascontiguousarray(
            b.reshape(-1, 128).T).astype(f32)

    return {
        "whp": pack_kxm(Wh_f),
        "wip": pack_kxm(Wi_f),
        "w1p": pack_kxm(W1_f),
        "w2p": pack_kxm(W2.astype(f32)),
        "bi": pack_bias(bi_f),
        "b1": pack_bias(b1_f),
        "b2": pack_bias(b2),
        "ident": np.eye(128, dtype=bf16),
    }


def kernel(x, ln1_scale, ln1_bias, Wi, Wh, b_lstm, ln2_scale, ln2_bias,
           W1, b1, W2, b2, _trace=False):
    x = np.asarray(x, np.float32)
    B, S, d = x.shape
    assert d == D and B % N_CORES == 0
    nc = _get_nc(S)
    weights = _prep_weights(
        np.asarray(ln1_scale, np.float32), np.asarray(ln1_bias, np.float32),
        np.asarray(Wi, np.float32), np.asarray(Wh, np.float32),
        np.asarray(b_lstm, np.float32), np.asarray(ln2_scale, np.float32),
        np.asarray(ln2_bias, np.float32), np.asarray(W1, np.float32),
        np.asarray(b1, np.float32), np.asarray(W2, np.float32),
        np.asarray(b2, np.float32))
    bl = B // N_CORES
    in_maps = []
    for c in range(N_CORES):
        m = dict(weights)
        m["xs"] = np.ascontiguousarray(x[c * bl:(c + 1) * bl])
        in_maps.append(m)
    res = run_bass_kernel_spmd(nc, in_maps, core_ids=list(range(N_CORES)),
                               trace=_trace)
    outs = [r["out"] for r in res.results]
    full = np.concatenate(outs, axis=0).astype(np.float32)
    if _trace:
        kernel._last_exec_time_ns = res.exec_time_ns
    return full
